# revision 1
# baseline (speedup 1.0000x reference)
"""Masked dense layer  out = tanh(x @ (w*mask_w) + b*mask_b)  on 8 TRN2 cores.

Data-parallel: x is sharded along the batch axis (32768 rows per core);
w/b/mask_w/mask_b are replicated. The HWDGE f32 slab stream runs at
~411-421 GB/s per core (vs the 435 GB/s SBUF-fabric ceiling; the 64 MiB/core
HBM read is mandatory traffic), so the kernel is built so that stream
free-runs and everything else hides behind it:

- Work per slab is split across DVE and ACT so neither engine paces the
  stream: DVE does half the rows with AFFINE_MUL_REDUCE (a 1x custom op,
  ~604ns/row) plus one f32 tensor_mul for the other half; ACT reduces the
  product rows via activation(Copy, accum_out=...) (~850ns/row) and
  applies Tanh(+bias) per chunk. (The v1 baseline ran all rows through AMR
  on DVE, which co-paced the stream and added a ~9us tail. An alternating
  1-AMR/3-ACT split measured 9us slower: the 3-row ACT bursts exceed the
  per-slab pace.)
- Chunk schedule [1,1,2, 4x62, 2,1,1]: tiny leading chunks because a DMA's
  completion semaphore lands ~5us after the bytes under a saturated fabric -
  small first slabs get DVE computing by ~14us instead of ~18us; 1 MiB
  middle slabs are the measured-fastest HWDGE shape (~2.5us/MiB; 2 MiB
  slabs measured 18% slower per byte); small tail chunks shorten the final
  dependency chain.
- Params load on the sync ring ahead of the slabs (issued later, their sems
  don't fire for ~10us); param math runs on DVE before chunk 0's slab sem
  arrives (GpSimd tensor ops trigger an 8us ucode LIBRARY_RELOAD mid-stream).
- No mid-stream output DMAs (they stall the slab stream ~5us each via
  shared DMA-completion semaphore lanes): one body write issued after all
  slab dma_starts, and the last 8 rows after the final Tanh.
"""

import numpy as np

import concourse.bacc as bacc
import concourse.bass as bass
import concourse.tile as tile
from concourse import mybir
from concourse.bass_utils import run_bass_kernel_spmd

N, F = 262144, 512
C = 8                 # cores
R = N // C            # rows per core  = 32768
P = 128               # SBUF partitions
RP = R // P           # rows per partition = 256
CHUNKS = [1, 1, 2] + [4] * 62 + [2, 1, 1]
assert sum(CHUNKS) == RP

_cached_nc = None


def build_bass() -> bass.Bass:
    nc = bacc.Bacc()

    x = nc.declare_dram_parameter("x", [R, F], mybir.dt.float32, isOutput=False)
    w = nc.declare_dram_parameter("w", [F, 1], mybir.dt.float32, isOutput=False)
    b = nc.declare_dram_parameter("b", [1], mybir.dt.float32, isOutput=False)
    mask_w = nc.declare_dram_parameter(
        "mask_w", [F, 1], mybir.dt.int32, isOutput=False
    )
    mask_b = nc.declare_dram_parameter("mask_b", [1], mybir.dt.int32, isOutput=False)
    out = nc.declare_dram_parameter("out", [R, 1], mybir.dt.float32, isOutput=True)

    # partition p <- rows [p*RP, (p+1)*RP)
    x_r = x[:, :].rearrange("(p r) f -> p r f", p=P)      # [128, 256, 512]
    out_r = out[:, :].rearrange("(p r) one -> p (r one)", p=P)  # [128, 256]

    def bcast(src_handle, count):
        """DRAM AP replicating a contiguous `count`-element vector across P partitions."""
        ap = src_handle[:]
        return bass.AP(tensor=ap.tensor, offset=ap.offset, ap=[[0, P], [1, count]])

    def rep_mid(ap2d, k):
        """View a [P, F] SBUF AP as [P, k, F] with 0-stride middle dim."""
        return bass.AP(
            tensor=ap2d.tensor,
            offset=ap2d.offset,
            ap=[ap2d.ap[0], [0, k], ap2d.ap[1]],
        )

    with tile.TileContext(nc) as tc:
        with (
            tc.tile_pool(name="singles", bufs=1) as singles,
            tc.tile_pool(name="slabs_big", bufs=16) as slabs_big,
            tc.tile_pool(name="slabs_small", bufs=2) as slabs_small,
            tc.tile_pool(name="prods", bufs=4) as prods,
            tc.tile_pool(name="vjunk", bufs=3) as vjunk,
            tc.tile_pool(name="ajunk", bufs=3) as ajunk,
            tc.tile_pool(name="stages", bufs=3) as stages,
        ):
            # only the wm params ride the sync ring ahead of the slab stream:
            # each dma_start costs ~0.66us of sequencer issue time, so the
            # bias params are issued mid-stream (after slab 2, absorbed by
            # queue depth) to start the stream ~1.3us earlier
            wb = singles.tile([P, F], mybir.dt.float32)
            nc.sync.dma_start(out=wb, in_=bcast(w, F))
            mwi = singles.tile([P, F], mybir.dt.int32)
            nc.sync.dma_start(out=mwi, in_=bcast(mask_w, F))

            # wm on DVE right away (ready before chunk 0's slab sem arrives)
            mw = singles.tile([P, F], mybir.dt.float32)
            nc.vector.tensor_copy(mw, mwi)  # i32 -> f32
            wm = singles.tile([P, F], mybir.dt.float32)
            nc.vector.tensor_mul(wm, wb, mw)

            bb = singles.tile([P, 1], mybir.dt.float32)
            mbi = singles.tile([P, 1], mybir.dt.int32)
            mb = singles.tile([P, 1], mybir.dt.float32)
            bm = singles.tile([P, 1], mybir.dt.float32)

            outt = singles.tile([P, RP], mybir.dt.float32)
            r0 = 0
            for ci, tr in enumerate(CHUNKS):
                half = tr // 2
                pool = slabs_big if tr == 4 else slabs_small
                slab = pool.tile([P, tr, F], mybir.dt.float32, tag=f"slab{tr}")
                nc.sync.dma_start(out=slab, in_=x_r[:, r0 : r0 + tr, :])
                stage = stages.tile([P, tr], mybir.dt.float32, tag=f"stage{tr}")
                # first half (rounded up) of the rows: fused mul+reduce on DVE
                n_amr = tr - half
                for t in range(n_amr):
                    junk = vjunk.tile([P, F], mybir.dt.bfloat16, tag="vj")
                    nc.vector.affine_mul_reduce(
                        out=junk,
                        accum_out=stage[:, t : t + 1],
                        in0=slab[:, t, :],
                        in1=wm,
                        scale=1.0,
                        bias=0.0,
                    )
                # second half: f32 multiply on DVE, reduce on ACT (f32 product:
                # same measured op costs as bf16, keeps rel err at ~6e-7)
                if half:
                    prod = prods.tile(
                        [P, half, F], mybir.dt.float32, tag=f"prod{half}"
                    )
                    nc.vector.tensor_mul(
                        prod, slab[:, n_amr:tr, :], rep_mid(wm[:, :], half)
                    )
                    for t in range(half):
                        aj = ajunk.tile([P, F], mybir.dt.bfloat16, tag="aj")
                        nc.scalar.activation(
                            out=aj,
                            in_=prod[:, t, :],
                            func=mybir.ActivationFunctionType.Copy,
                            accum_out=stage[:, n_amr + t : n_amr + t + 1],
                        )
                nc.scalar.activation(
                    out=outt[:, r0 : r0 + tr],
                    in_=stage,
                    func=mybir.ActivationFunctionType.Tanh,
                    bias=bm,
                    scale=1.0,
                )
                r0 += tr
                if ci == 4:
                    # bias param loads, issued mid-stream once the slab queue
                    # has a full slab of cushion (their sems land ~18us; the
                    # early Tanhs wait on bm but gate nothing until the body
                    # write at ~172us)
                    nc.sync.dma_start(out=bb, in_=bcast(b, 1))
                    nc.sync.dma_start(out=mbi, in_=bcast(mask_b, 1))
                if ci == 6:
                    # bias prep on DVE after chunk 6's ops: DVE executes in
                    # issue order, so this sits where the bias sems (~18us)
                    # are already fired and no chunk work stalls behind it
                    nc.vector.tensor_copy(mb, mbi)  # i32 -> f32
                    nc.vector.tensor_mul(bm, bb, mb)
            # issued after every slab dma_start (the sync ring is FIFO): the
            # body write drains while the tail chunks compute; the final 8
            # rows follow the last Tanh.
            nc.sync.dma_start(out=out_r[:, : RP - 4], in_=outt[:, : RP - 4])
            nc.sync.dma_start(out=out_r[:, RP - 4 :], in_=outt[:, RP - 4 :])

    nc.finalize()
    return nc


def run_sharded(inputs: dict, **run_kwargs):
    """Shard inputs, run on 8 cores, gather. Returns (output, BassKernelResults)."""
    global _cached_nc
    if _cached_nc is None:
        _cached_nc = build_bass()
    nc = _cached_nc

    x = np.ascontiguousarray(np.asarray(inputs["x"], dtype=np.float32))
    w = np.ascontiguousarray(np.asarray(inputs["w"], dtype=np.float32))
    b = np.ascontiguousarray(np.asarray(inputs["b"], dtype=np.float32))
    mask_w = np.ascontiguousarray(np.asarray(inputs["mask_w"], dtype=np.int32))
    mask_b = np.ascontiguousarray(np.asarray(inputs["mask_b"], dtype=np.int32))

    in_maps = [
        {
            "x": x[i * R : (i + 1) * R],
            "w": w,
            "b": b,
            "mask_w": mask_w,
            "mask_b": mask_b,
        }
        for i in range(C)
    ]
    res = run_bass_kernel_spmd(nc, in_maps, core_ids=list(range(C)), **run_kwargs)
    outs = [res.results[i]["out"] for i in range(C)]
    return np.concatenate(outs, axis=0), res


def kernel(x, w, b, mask_w, mask_b) -> np.ndarray:
    out, _ = run_sharded(
        {"x": x, "w": w, "b": b, "mask_w": mask_w, "mask_b": mask_b}
    )
    return out

